# revision 9
# baseline (speedup 1.0000x reference)
"""Trainium2 Bass kernel for Conv2D(sum of 20 1x1 convs) + QwenRMSNorm.

Math: y = einsum("bsi,loi->bso", x, conv_w) / L ; out = rmsnorm(y) * norm_w.
Since x does not depend on l, the 20-matrix contraction collapses to a single
matmul with W = sum_l conv_w[l] / L.  Host pre-sums/transposes/casts the weight
(one [H,H] matrix) and lays out x as token-sharded, hidden-major bf16 slabs;
the 8 NeuronCores each run matmul (bf16, fp32 accum) + RMSNorm on their 2048
tokens.  All device compute is token-local; no collectives.

Scheduling notes (v3):
  * Per-core HBM share is ~360 GB/s and both HWDGE rings (SP=sync,
    Act=scalar) split it, so the startup critical path is "first w chunk +
    first x chunk".  w streams in 8 fine chunks in exact consumption order
    on the scalar ring; the bulk x prefetch is queued BEHIND w on the same
    ring (FIFO) so it cannot steal bandwidth from the weight load.  Only
    x[tt0,tt1] + norm_w ride the sync ring early.
  * Output is bf16 (upcast on host): halves the output traffic and the
    end-of-kernel DMA drain.  Output DMAs ride the sync ring, which is idle
    after the first microseconds.
  * PE warm-up: the HAM clock gate keeps the PE at 1.2 GHz until it has been
    busy ~3.4us without gaps.  Dummy matmuls bridge the first-DMA wait.
  * x lives in one big SBUF tile; Tile's region tracker gives per-slice
    dependencies so matmuls only gate on the DMA stage that carries their
    slab.
"""

import numpy as np
import ml_dtypes
from contextlib import ExitStack

import concourse.bass as bass
import concourse.mybir as mybir
import concourse.tile as tile
from concourse.bass_utils import run_bass_kernel_spmd

N_CORES = 8
B, S, H, L = 4, 4096, 1024, 20
TOK = B * S               # 16384 tokens
TPC = TOK // N_CORES      # 2048 tokens per core
TB = TPC // 128           # 16 token-blocks of 128 per core
KB = H // 128             # 8 contraction blocks
NOH = H // 512            # 2 psum halves of the output row
XSL = KB * 128            # 1024 elements per (partition, token-block) of x
EPS = 1e-6

BF16 = mybir.dt.bfloat16
F32 = mybir.dt.float32
AF = mybir.ActivationFunctionType
OP = mybir.AluOpType

_BUILT = None       # cached Bass program
LAST_RESULTS = None  # BassKernelResults of the most recent run (for test harness)


def _legalize_multiwait(nc):
    """The walrus build here encodes exactly one semaphore wait per 64B
    instruction (NEURON_ISA_TPB_EVENTS has a single wait slot) and errors on
    Tile's multi-wait instructions.  Split surplus waits into standalone
    EVENT_SEMAPHORE instructions on the same engine, placed directly before
    the original instruction (same sequencer stream -> same semantics)."""
    n_ev = 0
    for f in nc.m.functions:
        for blk in f.blocks:
            insts = blk.instructions
            out = []
            changed = False
            for inst in list(insts):
                si = getattr(inst, "sync_info", None)
                waits = list(si.on_wait) if si is not None else []
                if len(waits) > 1:
                    changed = True
                    updates = list(si.on_update)
                    for w in waits[:-1]:
                        ev = mybir.InstEventSemaphore(
                            name=f"{inst.name}-sw{n_ev}", ins=[], outs=[])
                        n_ev += 1
                        ev.engine = inst.engine
                        ev.sync_info = mybir.SyncInfo(on_wait=[w], on_update=[])
                        out.append(ev)
                    inst.sync_info = mybir.SyncInfo(
                        on_wait=[waits[-1]], on_update=updates)
                out.append(inst)
            if changed:
                insts.clear()
                insts.extend(out)


def _build():
    nc = bass.Bass()
    # x^T slab layout per core: xt[tt, p, ib, t] = x[tt*128 + t, ib*128 + p], bf16
    xt_h = nc.dram_tensor("xt", [TB, 128, KB, 128], BF16, kind="ExternalInput")
    # weight layout: wt[p, ib, o] = W[o, ib*128 + p] with W = sum_l conv_w[l]/L, bf16
    wt_h = nc.dram_tensor("wt", [128, KB, H], BF16, kind="ExternalInput")
    nw_h = nc.dram_tensor("nw", [H], F32, kind="ExternalInput")
    out_h = nc.dram_tensor("out", [TPC, H], BF16, kind="ExternalOutput")

    with tile.TileContext(nc) as tc, ExitStack() as ctx:
        xpool = ctx.enter_context(tc.tile_pool(name="x", bufs=1))
        wpool = ctx.enter_context(tc.tile_pool(name="w", bufs=1))
        cpool = ctx.enter_context(tc.tile_pool(name="consts", bufs=1))
        opool = ctx.enter_context(tc.tile_pool(name="out", bufs=4))
        spool = ctx.enter_context(tc.tile_pool(name="scratch", bufs=2))
        stats = ctx.enter_context(tc.tile_pool(name="stats", bufs=8))
        psum = ctx.enter_context(tc.tile_pool(name="psum", bufs=4, space="PSUM"))

        w_sb = wpool.tile([128, KB, H], BF16)
        x_sb = xpool.tile([128, TB, KB, 128], BF16)
        G = 4  # token blocks computed w-chunk-major while the weights stream

        def x_dma(eng, a, b, ib0=0, ib1=KB):
            # x slabs [a, b) x contraction blocks [ib0, ib1) -> x_sb slice
            src = bass.AP(tensor=xt_h, offset=a * 128 * XSL + ib0 * 128,
                          ap=[[XSL, 128], [128 * XSL, b - a],
                              [1, (ib1 - ib0) * 128]])
            eng.dma_start(out=x_sb[:, a:b, ib0:ib1, :], in_=src)

        # sync ring: x for the phase-1 blocks in two ib-halves (matching the
        # order the w chunks are consumed), then norm_w; the output DMAs are
        # appended to this ring from the loops below.
        x_dma(nc.sync, 0, G, 0, KB // 2)
        x_dma(nc.sync, 0, G, KB // 2, KB)
        nw_sb = cpool.tile([128, H], F32)
        nc.sync.dma_start(
            out=nw_sb, in_=bass.AP(tensor=nw_h, offset=0, ap=[[0, 128], [1, H]]))

        # scalar ring: w in 8 chunks in exact consumption order (oh-major,
        # ib pairs), then the bulk x prefetch QUEUED BEHIND the weights so
        # it cannot compete with them for HBM bandwidth.
        for oh in range(NOH):
            for ibp in range(0, KB, 2):
                nc.scalar.dma_start(
                    out=w_sb[:, ibp:ibp + 2, oh * 512:(oh + 1) * 512],
                    in_=wt_h[:, ibp:ibp + 2, oh * 512:(oh + 1) * 512])
        x_dma(nc.scalar, G, 8)
        x_dma(nc.scalar, 8, 12)
        x_dma(nc.scalar, 12, 16)

        zero_sb = cpool.tile([128, 1], F32)
        nc.vector.memset(zero_sb, 0.0)
        eps_sb = cpool.tile([128, 1], F32)
        nc.vector.memset(eps_sb, EPS)

        # PE warm-up: dummy matmuls bridge the gap until the first w/x
        # chunks land, so the HAM clock gate sees uninterrupted activity
        # from well before the first real matmul and ramps the PE to
        # 2.4 GHz ~3.4us after the stream starts.  They write a region of
        # psum that tt0 overwrites later (start=True clears the bank), so
        # no extra PSUM bank is needed.
        dummy = cpool.tile([128, 128], BF16)
        nc.vector.memset(dummy, 0.0)
        yps = [psum.tile([128, H], F32, name="yp", tag="yp") for _ in range(G)]
        for _ in range(26):
            nc.tensor.matmul(yps[0][:, 0:128], dummy, dummy,
                             start=True, stop=True)

        sq = spool.tile([128, H], BF16)  # squares scratch, shared (write-only)

        def mk_square(yp, oh, half_sums):
            sl = slice(oh * 512, (oh + 1) * 512)
            nc.scalar.activation(out=sq[:, sl], in_=yp[:, sl],
                                 func=AF.Square, bias=zero_sb,
                                 accum_out=half_sums[:, oh:oh + 1])

        def finish_norm(tt, yp, half_sums, last=False):
            ssum = stats.tile([128, 1], F32)
            nc.vector.tensor_add(out=ssum, in0=half_sums[:, 0:1],
                                 in1=half_sums[:, 1:2])
            # std = sqrt(mean + eps); rstd = 1/std
            std = stats.tile([128, 1], F32)
            nc.scalar.activation(out=std, in_=ssum, func=AF.Sqrt,
                                 bias=eps_sb, scale=1.0 / H)
            rstd = stats.tile([128, 1], F32)
            nc.vector.reciprocal(out=rstd, in_=std)
            # out = (y * rstd) * norm_w, written bf16.  The last block is
            # split finer so its DMA starts as soon as possible.
            o_sb = opool.tile([128, H], BF16, name="o_sb", tag="o")
            nq = 4 if last else 2
            for q in range(nq):
                w = H // nq
                sl = slice(q * w, (q + 1) * w)
                nc.vector.scalar_tensor_tensor(
                    out=o_sb[:, sl], in0=yp[:, sl], scalar=rstd,
                    in1=nw_sb[:, sl], op0=OP.mult, op1=OP.mult,
                )
                if last:
                    nc.sync.dma_start(
                        out=out_h[tt * 128:(tt + 1) * 128, sl],
                        in_=o_sb[:, sl])
            if not last:
                nc.sync.dma_start(out=out_h[tt * 128:(tt + 1) * 128, :],
                                  in_=o_sb)

        # Phase 1: w-chunk-major over the first G token blocks.  Each
        # arriving 256KB w chunk feeds 2*G matmuls, so the PE's weight
        # demand (~600 GB/s tt-major) drops to ~150 GB/s, matching what HBM
        # can deliver while 8 cores all pull their weights.
        hs = {}
        for oh in range(NOH):
            for ibp in range(0, KB, 2):
                for t in range(G):
                    for ib in (ibp, ibp + 1):
                        nc.tensor.matmul(
                            yps[t][:, oh * 512:(oh + 1) * 512],
                            x_sb[:, t, ib, :],
                            w_sb[:, ib, oh * 512:(oh + 1) * 512],
                            start=(ib == 0),
                            stop=(ib == KB - 1),
                        )
            for t in range(G):
                if oh == 0:
                    hs[t] = stats.tile([128, 2], F32, name="hs", tag="hs")
                mk_square(yps[t], oh, hs[t])
        for t in range(G):
            finish_norm(t, yps[t], hs[t])

        # Phase 2: weights are resident; token-block-major.
        for tt in range(G, TB):
            yp = psum.tile([128, H], F32, name="yp", tag="yp")
            half_sums = stats.tile([128, 2], F32, name="hs", tag="hs")
            for oh in range(NOH):
                for ib in range(KB):
                    nc.tensor.matmul(
                        yp[:, oh * 512:(oh + 1) * 512],
                        x_sb[:, tt, ib, :],
                        w_sb[:, ib, oh * 512:(oh + 1) * 512],
                        start=(ib == 0),
                        stop=(ib == KB - 1),
                    )
                mk_square(yp, oh, half_sums)
            finish_norm(tt, yp, half_sums, last=(tt == TB - 1))

    _legalize_multiwait(nc)
    return nc


def host_prep(x, conv_w, norm_w):
    """Shard + lay out the full inputs into per-core device input maps."""
    bf16 = ml_dtypes.bfloat16

    # Collapse the 20 1x1 convs: W[o,i] = sum_l conv_w[l,o,i] / L
    w = np.asarray(conv_w).sum(axis=0) * (1.0 / L)          # [H(o), H(i)] f32
    # wt[p, ib, o] = W[o, ib*128+p]
    wt = np.ascontiguousarray(
        w.reshape(H, KB, 128).transpose(2, 1, 0).astype(bf16))
    nw = np.ascontiguousarray(np.asarray(norm_w), dtype=np.float32)

    x2d = np.asarray(x).reshape(TOK, H)
    xbf = x2d.astype(bf16)

    in_maps = []
    for c in range(N_CORES):
        xc = xbf[c * TPC:(c + 1) * TPC]                      # [TPC, H]
        # xt[tt, p, ib, t] = xc[tt*128+t, ib*128+p]
        xtc = np.ascontiguousarray(
            xc.reshape(TB, 128, KB, 128).transpose(0, 3, 2, 1))
        in_maps.append({"xt": xtc, "wt": wt, "nw": nw})
    return in_maps


def kernel(x, conv_w, norm_w):
    global _BUILT, LAST_RESULTS
    if _BUILT is None:
        _BUILT = _build()
    nc = _BUILT

    x = np.asarray(x)
    out_dtype = x.dtype
    in_maps = host_prep(x, conv_w, norm_w)

    res = run_bass_kernel_spmd(nc, in_maps, core_ids=list(range(N_CORES)))
    LAST_RESULTS = res

    out = np.concatenate([r["out"] for r in res.results], axis=0)
    return out.reshape(B, S, H).astype(out_dtype, copy=False)


# revision 12
# speedup vs baseline: 1.0294x; 1.0294x over previous
"""Trainium2 Bass kernel for Conv2D(sum of 20 1x1 convs) + QwenRMSNorm.

Math: y = einsum("bsi,loi->bso", x, conv_w) / L ; out = rmsnorm(y) * norm_w.
Since x does not depend on l, the 20-matrix contraction collapses to a single
matmul with W = sum_l conv_w[l] / L.  Host pre-sums/transposes/casts the weight
(one [H,H] matrix) and lays out x as token-sharded, hidden-major bf16 slabs;
the 8 NeuronCores each run matmul (bf16, fp32 accum) + RMSNorm on their 2048
tokens.  All device compute is token-local; no collectives.

Scheduling notes (v3):
  * Per-core HBM share is ~360 GB/s and both HWDGE rings (SP=sync,
    Act=scalar) split it, so the startup critical path is "first w chunk +
    first x chunk".  w streams in 8 fine chunks in exact consumption order
    on the scalar ring; the bulk x prefetch is queued BEHIND w on the same
    ring (FIFO) so it cannot steal bandwidth from the weight load.  Only
    x[tt0,tt1] + norm_w ride the sync ring early.
  * Output is bf16 (upcast on host): halves the output traffic and the
    end-of-kernel DMA drain.  Output DMAs ride the sync ring, which is idle
    after the first microseconds.
  * PE warm-up: the HAM clock gate keeps the PE at 1.2 GHz until it has been
    busy ~3.4us without gaps.  Dummy matmuls bridge the first-DMA wait.
  * x lives in one big SBUF tile; Tile's region tracker gives per-slice
    dependencies so matmuls only gate on the DMA stage that carries their
    slab.
"""

import numpy as np
import ml_dtypes
from contextlib import ExitStack

import concourse.bass as bass
import concourse.mybir as mybir
import concourse.tile as tile
from concourse.bass_utils import run_bass_kernel_spmd

N_CORES = 8
B, S, H, L = 4, 4096, 1024, 20
TOK = B * S               # 16384 tokens
TPC = TOK // N_CORES      # 2048 tokens per core
TB = TPC // 128           # 16 token-blocks of 128 per core
KB = H // 128             # 8 contraction blocks
NOH = H // 512            # 2 psum halves of the output row
XSL = KB * 128            # 1024 elements per (partition, token-block) of x
EPS = 1e-6

BF16 = mybir.dt.bfloat16
F32 = mybir.dt.float32
AF = mybir.ActivationFunctionType
OP = mybir.AluOpType

_BUILT = None       # cached Bass program
LAST_RESULTS = None  # BassKernelResults of the most recent run (for test harness)


def _legalize_multiwait(nc):
    """The walrus build here encodes exactly one semaphore wait per 64B
    instruction (NEURON_ISA_TPB_EVENTS has a single wait slot) and errors on
    Tile's multi-wait instructions.  Split surplus waits into standalone
    EVENT_SEMAPHORE instructions on the same engine, placed directly before
    the original instruction (same sequencer stream -> same semantics)."""
    n_ev = 0
    for f in nc.m.functions:
        for blk in f.blocks:
            insts = blk.instructions
            out = []
            changed = False
            for inst in list(insts):
                si = getattr(inst, "sync_info", None)
                waits = list(si.on_wait) if si is not None else []
                if len(waits) > 1:
                    changed = True
                    updates = list(si.on_update)
                    for w in waits[:-1]:
                        ev = mybir.InstEventSemaphore(
                            name=f"{inst.name}-sw{n_ev}", ins=[], outs=[])
                        n_ev += 1
                        ev.engine = inst.engine
                        ev.sync_info = mybir.SyncInfo(on_wait=[w], on_update=[])
                        out.append(ev)
                    inst.sync_info = mybir.SyncInfo(
                        on_wait=[waits[-1]], on_update=updates)
                out.append(inst)
            if changed:
                insts.clear()
                insts.extend(out)


def _build():
    nc = bass.Bass()
    # x^T slab layout per core: xt[tt, p, ib, t] = x[tt*128 + t, ib*128 + p], bf16
    xt_h = nc.dram_tensor("xt", [TB, 128, KB, 128], BF16, kind="ExternalInput")
    # weight layout: wt[p, ib, o] = W[o, ib*128 + p] with W = sum_l conv_w[l]/L, bf16
    wt_h = nc.dram_tensor("wt", [128, KB, H], BF16, kind="ExternalInput")
    nw_h = nc.dram_tensor("nw", [H], F32, kind="ExternalInput")
    out_h = nc.dram_tensor("out", [TPC, H], BF16, kind="ExternalOutput")

    with tile.TileContext(nc) as tc, ExitStack() as ctx:
        xpool = ctx.enter_context(tc.tile_pool(name="x", bufs=1))
        wpool = ctx.enter_context(tc.tile_pool(name="w", bufs=1))
        cpool = ctx.enter_context(tc.tile_pool(name="consts", bufs=1))
        opool = ctx.enter_context(tc.tile_pool(name="out", bufs=4))
        spool = ctx.enter_context(tc.tile_pool(name="scratch", bufs=2))
        stats = ctx.enter_context(tc.tile_pool(name="stats", bufs=8))
        psum = ctx.enter_context(tc.tile_pool(name="psum", bufs=4, space="PSUM"))

        w_sb = wpool.tile([128, KB, H], BF16)
        x_sb = xpool.tile([128, TB, KB, 128], BF16)
        G = 2  # token blocks computed w-chunk-major while the weights stream

        def x_dma(eng, a, b, ib0=0, ib1=KB):
            # x slabs [a, b) x contraction blocks [ib0, ib1) -> x_sb slice
            src = bass.AP(tensor=xt_h, offset=a * 128 * XSL + ib0 * 128,
                          ap=[[XSL, 128], [128 * XSL, b - a],
                              [1, (ib1 - ib0) * 128]])
            eng.dma_start(out=x_sb[:, a:b, ib0:ib1, :], in_=src)

        # sync ring: x for the first blocks in ib-halves (matching the order
        # the w chunks are consumed), then x[2:4] and norm_w; the output
        # DMAs are appended to this ring from the loops below.
        x_dma(nc.sync, 0, G, 0, KB // 2)
        x_dma(nc.sync, 0, G, KB // 2, KB)
        x_dma(nc.sync, G, 4)
        nw_sb = cpool.tile([128, H], F32)
        nc.sync.dma_start(
            out=nw_sb, in_=bass.AP(tensor=nw_h, offset=0, ap=[[0, 128], [1, H]]))

        # scalar ring: w in 8 chunks in exact consumption order (oh-major,
        # ib pairs), then the bulk x prefetch QUEUED BEHIND the weights so
        # it cannot compete with them for HBM bandwidth.
        for oh in range(NOH):
            for ibp in range(0, KB, 2):
                nc.scalar.dma_start(
                    out=w_sb[:, ibp:ibp + 2, oh * 512:(oh + 1) * 512],
                    in_=wt_h[:, ibp:ibp + 2, oh * 512:(oh + 1) * 512])
        x_dma(nc.scalar, 4, 8)
        x_dma(nc.scalar, 8, 12)
        x_dma(nc.scalar, 12, 16)

        zero_sb = cpool.tile([128, 1], F32)
        nc.vector.memset(zero_sb, 0.0)
        eps_sb = cpool.tile([128, 1], F32)
        nc.vector.memset(eps_sb, EPS)

        # PE warm-up: dummy matmuls bridge the gap until the first w/x
        # chunks land, so the HAM clock gate sees uninterrupted activity
        # from well before the first real matmul and ramps the PE to
        # 2.4 GHz ~3.4us after the stream starts.  They write regions of
        # psum that later blocks overwrite (start=True clears the bank), so
        # no extra PSUM bank is needed.
        dummy = cpool.tile([128, 512], BF16)
        nc.vector.memset(dummy, 0.0)
        yps = [psum.tile([128, H], F32, name="yp", tag="yp") for _ in range(G)]
        warm_ps = psum.tile([128, H], F32, name="yp", tag="yp")  # = tt2's slot
        for _ in range(13):
            nc.tensor.matmul(warm_ps[:, 0:512], dummy[:, 0:128], dummy,
                             start=True, stop=True)

        sq = spool.tile([128, H], BF16)  # squares scratch, shared (write-only)

        def mk_square(yp, oh, half_sums):
            sl = slice(oh * 512, (oh + 1) * 512)
            nc.scalar.activation(out=sq[:, sl], in_=yp[:, sl],
                                 func=AF.Square, bias=zero_sb,
                                 accum_out=half_sums[:, oh:oh + 1])

        def finish_norm(tt, yp, half_sums, last=False):
            ssum = stats.tile([128, 1], F32)
            nc.vector.tensor_add(out=ssum, in0=half_sums[:, 0:1],
                                 in1=half_sums[:, 1:2])
            # std = sqrt(mean + eps); rstd = 1/std
            std = stats.tile([128, 1], F32)
            nc.scalar.activation(out=std, in_=ssum, func=AF.Sqrt,
                                 bias=eps_sb, scale=1.0 / H)
            rstd = stats.tile([128, 1], F32)
            nc.vector.reciprocal(out=rstd, in_=std)
            # out = (y * rstd) * norm_w, written bf16.  The last block is
            # split finer so its DMA starts as soon as possible.
            o_sb = opool.tile([128, H], BF16, name="o_sb", tag="o")
            nq = 4 if last else 2
            for q in range(nq):
                w = H // nq
                sl = slice(q * w, (q + 1) * w)
                nc.vector.scalar_tensor_tensor(
                    out=o_sb[:, sl], in0=yp[:, sl], scalar=rstd,
                    in1=nw_sb[:, sl], op0=OP.mult, op1=OP.mult,
                )
                if last:
                    nc.sync.dma_start(
                        out=out_h[tt * 128:(tt + 1) * 128, sl],
                        in_=o_sb[:, sl])
            if not last:
                nc.sync.dma_start(out=out_h[tt * 128:(tt + 1) * 128, :],
                                  in_=o_sb)

        # Phase 1: w-chunk-major over the first G token blocks.  Each
        # arriving 256KB w chunk feeds 2*G matmuls, so the PE's weight
        # demand drops to what HBM can deliver while 8 cores all pull their
        # weights.  Dummy matmuls pad the arrival gaps so the HAM clock
        # gate sees continuous activity and ramps to 2.4 GHz early.
        hs = {}
        for oh in range(NOH):
            for ibp in range(0, KB, 2):
                for t in range(G):
                    for ib in (ibp, ibp + 1):
                        nc.tensor.matmul(
                            yps[t][:, oh * 512:(oh + 1) * 512],
                            x_sb[:, t, ib, :],
                            w_sb[:, ib, oh * 512:(oh + 1) * 512],
                            start=(ib == 0),
                            stop=(ib == KB - 1),
                        )
                if oh == 0 or ibp < KB - 2:  # no filler after the last chunk
                    for _ in range(3):
                        nc.tensor.matmul(warm_ps[:, 0:512], dummy[:, 0:128],
                                         dummy, start=True, stop=True)
            for t in range(G):
                if oh == 0:
                    hs[t] = stats.tile([128, 2], F32, name="hs", tag="hs")
                mk_square(yps[t], oh, hs[t])
        for t in range(G):
            finish_norm(t, yps[t], hs[t])

        # Phase 2: weights are resident; token-block-major.
        for tt in range(G, TB):
            yp = psum.tile([128, H], F32, name="yp", tag="yp")
            half_sums = stats.tile([128, 2], F32, name="hs", tag="hs")
            for oh in range(NOH):
                for ib in range(KB):
                    nc.tensor.matmul(
                        yp[:, oh * 512:(oh + 1) * 512],
                        x_sb[:, tt, ib, :],
                        w_sb[:, ib, oh * 512:(oh + 1) * 512],
                        start=(ib == 0),
                        stop=(ib == KB - 1),
                    )
                mk_square(yp, oh, half_sums)
            finish_norm(tt, yp, half_sums, last=(tt == TB - 1))

    _legalize_multiwait(nc)
    return nc


def host_prep(x, conv_w, norm_w):
    """Shard + lay out the full inputs into per-core device input maps."""
    bf16 = ml_dtypes.bfloat16

    # Collapse the 20 1x1 convs: W[o,i] = sum_l conv_w[l,o,i] / L
    w = np.asarray(conv_w).sum(axis=0) * (1.0 / L)          # [H(o), H(i)] f32
    # wt[p, ib, o] = W[o, ib*128+p]
    wt = np.ascontiguousarray(
        w.reshape(H, KB, 128).transpose(2, 1, 0).astype(bf16))
    nw = np.ascontiguousarray(np.asarray(norm_w), dtype=np.float32)

    x2d = np.asarray(x).reshape(TOK, H)
    xbf = x2d.astype(bf16)

    in_maps = []
    for c in range(N_CORES):
        xc = xbf[c * TPC:(c + 1) * TPC]                      # [TPC, H]
        # xt[tt, p, ib, t] = xc[tt*128+t, ib*128+p]
        xtc = np.ascontiguousarray(
            xc.reshape(TB, 128, KB, 128).transpose(0, 3, 2, 1))
        in_maps.append({"xt": xtc, "wt": wt, "nw": nw})
    return in_maps


def kernel(x, conv_w, norm_w):
    global _BUILT, LAST_RESULTS
    if _BUILT is None:
        _BUILT = _build()
    nc = _BUILT

    x = np.asarray(x)
    out_dtype = x.dtype
    in_maps = host_prep(x, conv_w, norm_w)

    res = run_bass_kernel_spmd(nc, in_maps, core_ids=list(range(N_CORES)))
    LAST_RESULTS = res

    out = np.concatenate([r["out"] for r in res.results], axis=0)
    return out.reshape(B, S, H).astype(out_dtype, copy=False)
